# revision 21
# baseline (speedup 1.0000x reference)
"""Trainium2 Bass kernel for nn_AMHSA (dense transformer block, B=2, C=256, 64x64).

Sharding: 8 cores, zero collectives. Core c handles batch b=c//4 and query
slice s=c%4 (1024 of 4096 spatial tokens). Each core in a batch group
redundantly computes the front (token embed + multi-scale dilated-conv bias +
K/V over all 4096 tokens), then attention for all 4 heads over its own
1024-query slice, projection, and the final residual for its slice. The
per-core query offset arrives as an int32 input applied through PE/DVE
register-offset (dynamic) access patterns, so all 8 cores run one SPMD
program.

All matmuls run in bf16 (fp32 PSUM accumulation). Weights are pre-packed,
pre-transposed, and concatenated host-side in pack_inputs(). The multi-scale
stack rows use a ci-major layout (row = c*9 + j) so the im2col copies and
rotation placements are single merged DMAs; the fuse weights are permuted to
match on the host. Softmax exp is split between the scalar engine (true exp)
and the vector engine (int16-bitcast bf16 fast exp, ~2-3% per-element, which
washes out over 4096-key softmax sums).
"""
import math
import numpy as np
import ml_dtypes
import concourse.bass as bass
import concourse.mybir as mybir
import concourse.tile as tile
from concourse import bacc
from concourse.bass_utils import run_bass_kernel_spmd

F32 = mybir.dt.float32
BF16 = mybir.dt.bfloat16
I32 = mybir.dt.int32
I16 = mybir.dt.int16
BF = ml_dtypes.bfloat16

NH, DH = 4, 64
CMID = 14
DILS = (1, 2, 3)
N_TOK = 4096
Q = 1024
NKC = N_TOK // 128
NJ = N_TOK // 512
AluOp = mybir.AluOpType
ActFn = mybir.ActivationFunctionType

# fast bf16 exp: bitcast(int16(round(x*FEXP_S + FEXP_B))) ~= exp(x*0.125)
FEXP_S = 0.125 * 128.0 / math.log(2.0)
FEXP_B = 127.0 * 128.0 - 5.6

# wcat column layout [256, 1550]:
# wtokT 0:256 | wqT 256:512 | wkT 512:768 | wvT 768:1024 | wprojT 1024:1280 |
# wfeat 1280:1536 | wredT 1536:1550


def build(debug=False):
    nc = bacc.Bacc("TRN2", target_bir_lowering=False, debug=False, num_devices=8)

    def param(name, shape, dt=BF16):
        return nc.declare_dram_parameter(name, list(shape), dt, isOutput=False)

    P = {}
    P["xbf"] = param("xbf", [256, N_TOK])
    P["xslice"] = param("xslice", [256, Q], F32)
    P["qoff"] = param("qoff", [1, 1], I32)
    P["wcat"] = param("wcat", [256, 1550])
    P["Lcat"] = param("Lcat", [127, 504])          # L1a|L1b|L2a|L2b
    P["wfusecat"] = param("wfusecat", [127, 512])  # [:,0:256]=wfuseT2, [0:126,256:512]=wfuseT1
    P["bcat"] = param("bcat", [128, 3], F32)       # btok0|btok1|bred(rows 0:14)
    P["bfeat_bf"] = param("bfeat_bf", [1, 256])
    P["out_ext"] = nc.declare_dram_parameter("out", [256, Q], F32, isOutput=True)
    dbg = {}
    if debug:
        for nm, shp, dt in [
            ("d_token_new", [256, N_TOK], BF16),
            ("d_kp0", [128, N_TOK], BF16),
            ("d_v1", [128, NKC * 4 * 65], BF16),
            ("d_q0", [128, Q], BF16),
            ("d_out_all", [256, Q], BF16),
        ]:
            dbg[nm] = nc.declare_dram_parameter(nm, shp, dt, isOutput=True)

    with tile.TileContext(nc) as tc:
        _emit(nc, tc, P, dbg)
    nc.compile()
    return nc


def _raw(t, offset, dims):
    return bass.AP(t[:].tensor, offset, [list(d) for d in dims])


def _emit(nc, tc, P, dbg):
    out_ext = P["out_ext"]

    import contextlib
    ctx = contextlib.ExitStack()
    with ctx:
        pers = ctx.enter_context(tc.tile_pool(name="pers", bufs=1))

        # ---------- persistent tiles ----------
        xsl = [pers.tile([128, Q], F32, tag=f"xsl{i}", name=f"xsl{i}") for i in range(2)]
        tok_new = [pers.tile([128, N_TOK], BF16, tag=f"tokn{i}", name=f"tokn{i}") for i in range(2)]
        kp = [pers.tile([128, N_TOK], BF16, tag=f"kp{i}", name=f"kp{i}") for i in range(2)]
        v1 = pers.tile([128, NKC, 4 * 65], BF16, tag="v1", name="v1")
        qsb = [pers.tile([128, Q], BF16, tag=f"q{i}", name=f"q{i}") for i in range(2)]
        out_all = [pers.tile([128, Q], BF16, tag=f"oa{i}", name=f"oa{i}") for i in range(2)]
        tok_upd = [pers.tile([128, Q], BF16, tag=f"tu{i}", name=f"tu{i}") for i in range(2)]
        qt = pers.tile([1, 1], I32, tag="qt", name="qt")
        ones4k = pers.tile([1, 4480], BF16, tag="ones4k", name="ones4k")

        wcat_t = [pers.tile([128, 1550], BF16, tag=f"wcat{k}", name=f"wcat{k}") for k in range(2)]
        Lcat_t = pers.tile([127, 504], BF16, tag="Lcat", name="Lcat")
        wfuse_t = pers.tile([127, 512], BF16, tag="wfuse", name="wfuse")
        bcat_t = pers.tile([128, 3], F32, tag="bcat", name="bcat")
        bfeat_t = pers.tile([1, 256], BF16, tag="bfeat", name="bfeat")

        def wsl(k, lo, hi):
            return wcat_t[k][:, lo:hi]

        for k in range(2):
            r = slice(128 * k, 128 * (k + 1))
            nc.sync.dma_start(wcat_t[k][:], P["wcat"].ap()[r, :])
            nc.sync.dma_start(xsl[k][:], P["xslice"].ap()[r, :])
        nc.sync.dma_start(Lcat_t[:], P["Lcat"].ap())
        nc.sync.dma_start(wfuse_t[:], P["wfusecat"].ap())
        nc.sync.dma_start(bcat_t[:], P["bcat"].ap())
        nc.sync.dma_start(bfeat_t[:], P["bfeat_bf"].ap())
        nc.sync.dma_start(qt[:], P["qoff"].ap())
        nc.gpsimd.memset(ones4k[:], 1.0)
        v1h = v1[:].rearrange("p c (h e) -> p c h e", e=65)
        nc.gpsimd.memset(v1h[:, :, :, 64:65], 1.0)

        btok_t = [bcat_t[:, k:k + 1] for k in range(2)]
        bred_t = bcat_t[0:CMID, 2:3]

        # per-core query offset registers
        pe_reg = nc.alloc_register(mybir.EngineType.PE, "qoff_pe")
        dve_reg = nc.alloc_register(mybir.EngineType.DVE, "qoff_dve")
        nc.tensor.reg_load(pe_reg, qt[0:1, 0:1])
        nc.vector.reg_load(dve_reg, qt[0:1, 0:1])
        pe_off = nc.snap(pe_reg, min_val=0, max_val=N_TOK - Q, donate=True)
        dve_off = nc.snap(dve_reg, min_val=0, max_val=N_TOK - Q, donate=True)

        with (
            tc.tile_pool(name="front", bufs=1) as front,
            tc.tile_pool(name="psA", bufs=6, space="PSUM") as psA,
        ):
            tok_bf = [front.tile([128, N_TOK], BF16, tag=f"tokbf{i}", name=f"tokbf{i}") for i in range(2)]
            z_pad = front.tile([CMID, 72, 70], BF16, tag="z_pad", name="z_pad")
            rotz = [front.tile([126, N_TOK], BF16, tag=f"rotz{i}", name=f"rotz{i}") for i in range(2)]
            feats1 = front.tile([126, N_TOK], BF16, tag="feats1", name="feats1")
            feats2 = front.tile([127, N_TOK], BF16, tag="feats2", name="feats2")

            # ---- stage A: token = wtok @ x + btok (copies on ACT) ----
            with tc.tile_pool(name="xpool", bufs=1) as xpool:
                xbf = [xpool.tile([128, N_TOK], BF16, tag=f"xbf{i}", name=f"xbf{i}") for i in range(2)]
                for k in range(2):
                    for cq in range(2):
                        cs = slice(2048 * cq, 2048 * (cq + 1))
                        nc.sync.dma_start(xbf[k][:, cs],
                                          P["xbf"].ap()[128 * k:128 * (k + 1), cs])
                for j in range(NJ):
                    js = slice(512 * j, 512 * (j + 1))
                    for m in range(2):
                        pt = psA.tile([128, 512], F32, tag="ps", name="ps")
                        for k in range(2):
                            nc.tensor.matmul(pt[:], wsl(k, 128 * m, 128 * (m + 1)),
                                             xbf[k][:, js], start=(k == 0), stop=(k == 1))
                        if m == 0:
                            nc.scalar.activation(tok_bf[m][:, js], pt[:],
                                                 ActFn.Identity, bias=btok_t[m])
                        else:
                            nc.vector.tensor_scalar_add(tok_bf[m][:, js], pt[:],
                                                        btok_t[m])

            # ---- stage B: z (zero-padded) + rotations ----
            nc.gpsimd.memset(z_pad[:], 0.0)
            for j in range(NJ):
                js = slice(512 * j, 512 * (j + 1))
                pz = psA.tile([CMID, 512], F32, tag="ps", name="ps")
                for k in range(2):
                    nc.tensor.matmul(pz[:], wsl(k, 1536, 1550), tok_bf[k][:, js],
                                     start=(k == 0), stop=(k == 1))
                zdst = z_pad[:, 4 + 8 * j:4 + 8 * (j + 1), 3:67]
                zsrc = pz[:].rearrange("c (a b) -> c a b", b=64)
                if j % 2 == 0:
                    nc.scalar.activation(zdst, zsrc, ActFn.Identity, bias=bred_t)
                else:
                    nc.vector.tensor_scalar_add(zdst, zsrc, bred_t)

            z3 = z_pad[:, 4:68, 3:67]
            rots = [
                z3,                                      # r0
                z3[:, :, ::-1],                          # r1 flip W
                z3[:, ::-1, :],                          # r2 flip H
                z3[:, :, ::-1].transpose([0, 2, 1]),     # r3 rot90
                z3[:, ::-1, ::-1],                       # r4 rot180
                z3.transpose([0, 2, 1])[:, :, ::-1],     # r5 rot270
            ]
            # rotz rows are j-major (row = j*14 + c): plain block placements
            for r in range(6):
                rt = front.tile([CMID, 64, 64], BF16, tag="rt",
                                name=f"rt{r}", bufs=2)
                if r % 3 == 0:
                    nc.vector.tensor_copy(rt[:], rots[r])
                elif r % 3 == 1:
                    nc.scalar.copy(rt[:], rots[r])
                else:
                    nc.gpsimd.tensor_copy(rt[:], rots[r])
                nc.sync.dma_start(rotz[0][14 * r:14 * (r + 1), :],
                                  rt[:].rearrange("c h w -> c (h w)"))
            for cq in range(4):
                cs = slice(1024 * cq, 1024 * (cq + 1))
                nc.sync.dma_start(rotz[0][84:126, cs], rotz[0][0:42, cs])
                nc.sync.dma_start(rotz[1][0:42, cs], rotz[0][42:84, cs])
                nc.sync.dma_start(rotz[1][42:126, cs], rotz[0][0:84, cs])

            # ---- stages C+D: im2col (one merged DMA each) + convs + feats ----
            ZPP = 72 * 70

            def build_z9(di):
                d = DILS[di]
                t = front.tile([127, 64, 70], BF16, tag="z9",
                               name=f"z9_{di}", bufs=2)
                tf = t[:].rearrange("k h w -> k (h w)")
                for kh in range(3):
                    srcp = _raw(z_pad, (4 + d * (kh - 1)) * 70 - d,
                                [[ZPP, CMID], [d, 3], [1, 4480]])
                    dstp = _raw(t, kh * 42 * 4480, [[4480, 42], [1, 4480]])
                    nc.sync.dma_start(dstp, srcp)
                nc.sync.dma_start(tf[126:127, :], ones4k[:])
                return t

            def emit_stack(fdst, la, lb, za, zb, rz):
                for j in range(NJ):
                    js = slice(512 * j, 512 * (j + 1))
                    py = psA.tile([126, 512], F32, tag="ps", name="py")
                    nc.tensor.matmul(py[:], Lcat_t[:, 126 * la:126 * (la + 1)],
                                     za[:][:, 8 * j:8 * (j + 1), 3:67],
                                     start=True, stop=False)
                    nc.tensor.matmul(py[:], Lcat_t[:, 126 * lb:126 * (lb + 1)],
                                     zb[:][:, 8 * j:8 * (j + 1), 3:67],
                                     start=False, stop=True)
                    nc.vector.tensor_mul(fdst[0:126, js], py[:], rz[:, js])

            z9_0 = build_z9(0)
            z9_1 = build_z9(1)
            emit_stack(feats1, 0, 1, z9_0, z9_1, rotz[0])
            z9_2 = build_z9(2)
            emit_stack(feats2, 2, 3, z9_1, z9_2, rotz[1])
            nc.sync.dma_start(feats2[126:127, :], ones4k[:, 0:N_TOK])

            # ---- stage E: fuse conv + residual -> token_new ----
            for m in range(2):
                ms = slice(128 * m, 128 * (m + 1))
                ms2 = slice(256 + 128 * m, 256 + 128 * (m + 1))
                for j in range(NJ):
                    js = slice(512 * j, 512 * (j + 1))
                    pf = psA.tile([128, 512], F32, tag="ps", name="ps")
                    nc.tensor.matmul(pf[:], wfuse_t[0:126, ms2], feats1[:, js],
                                     start=True, stop=False)
                    nc.tensor.matmul(pf[:], wfuse_t[:, ms], feats2[:, js],
                                     start=False, stop=True)
                    nc.vector.tensor_add(tok_new[m][:, js], pf[:], tok_bf[m][:, js])

            # ---- stage F: K pairs, V^T, Q ----
            for p in range(2):
                lo = 512 + 128 * p
                for j in range(NJ):
                    js = slice(512 * j, 512 * (j + 1))
                    pk = psA.tile([128, 512], F32, tag="ps", name="ps")
                    for k in range(2):
                        nc.tensor.matmul(pk[:], wsl(k, lo, lo + 128),
                                         tok_new[k][:, js], start=(k == 0), stop=(k == 1))
                    if p == 0:
                        nc.scalar.copy(kp[p][:, js], pk[:])
                    else:
                        nc.vector.tensor_copy(kp[p][:, js], pk[:])
            for t in range(NKC):
                ts_ = slice(128 * t, 128 * (t + 1))
                pv = psA.tile([128, 256], F32, tag="ps", name="ps")
                for k in range(2):
                    nc.tensor.matmul(pv[:], tok_new[k][:, ts_], wsl(k, 768, 1024),
                                     start=(k == 0), stop=(k == 1))
                pv4 = pv[:].rearrange("p (h e) -> p h e", e=64)
                nc.vector.tensor_copy(v1h[:, t, :, 0:64], pv4)
            for p in range(2):
                lo = 256 + 128 * p
                for j in range(2):
                    pq = psA.tile([128, 512], F32, tag="ps", name="ps")
                    for k in range(2):
                        rhs = tok_new[k][:, bass.ds(pe_off, Q)][:, 512 * j:512 * (j + 1)]
                        nc.tensor.matmul(pq[:], wsl(k, lo, lo + 128), rhs,
                                         start=(k == 0), stop=(k == 1))
                    nc.scalar.copy(qsb[p][:, 512 * j:512 * (j + 1)], pq[:])

        # ---------- attention ----------
        # Two passes per head pair, one per 512-query half: av PSUM shrinks to
        # one bank per head, freeing room for 3-deep att PSUM buffering so the
        # PE streams without gating on the exp engines.
        with (
            tc.tile_pool(name="attps", bufs=3, space="PSUM") as attps,
            tc.tile_pool(name="avps", bufs=1, space="PSUM") as avps,
            tc.tile_pool(name="expp", bufs=4) as expp,
            tc.tile_pool(name="normp", bufs=1) as normp,
        ):
            for p in range(2):
                for j in range(2):
                    js = slice(512 * j, 512 * (j + 1))
                    av = [avps.tile([65, 512], F32, tag=f"av{h}", name=f"av{h}")
                          for h in range(2)]
                    for kc in range(NKC):
                        kcs = slice(128 * kc, 128 * (kc + 1))
                        at = [None, None]
                        for h in range(2):
                            hr = slice(64 * h, 64 * (h + 1))
                            a = attps.tile([128, 512], F32, tag=f"at{h}", name=f"at{h}")
                            nc.tensor.matmul(a[:], kp[p][hr, kcs],
                                             qsb[p][hr, js], start=True, stop=True)
                            at[h] = a
                        e0 = expp.tile([128, 512], BF16, tag="e0", name="e0")
                        nc.scalar.activation(e0[:], at[0][:], ActFn.Exp, scale=0.125)
                        e1 = expp.tile([128, 512], I16, tag="e1", name="e1")
                        nc.vector.tensor_scalar(e1[:], at[1][:], FEXP_S, FEXP_B,
                                                AluOp.mult, AluOp.add)
                        ex = [e0[:], e1[:].bitcast(BF16)]
                        for h in range(2):
                            head = 2 * p + h
                            nc.tensor.matmul(
                                av[h][:], v1h[:, kc, head, :], ex[h],
                                start=(kc == 0), stop=(kc == NKC - 1))
                    # normalize this query half (broadcast on idle gpsimd)
                    for h in range(2):
                        r_f = normp.tile([1, 512], F32, tag="r_f", name="r_f")
                        r_bf = normp.tile([1, 512], BF16, tag="r_bf", name="r_bf")
                        bc_sb = normp.tile([64, 512], BF16, tag="bc_sb", name="bc_sb")
                        nc.vector.reciprocal(r_f[:], av[h][64:65, :])
                        nc.vector.tensor_copy(r_bf[:], r_f[:])
                        nc.gpsimd.partition_broadcast(bc_sb[:], r_bf[:])
                        nc.vector.tensor_mul(
                            out_all[p][64 * h:64 * (h + 1), js],
                            av[h][0:64, :], bc_sb[:])

            # ---------- tail: proj + residual + feat + output ----------
            for m in range(2):
                lo = 1024 + 128 * m
                for j in range(2):
                    js = slice(512 * j, 512 * (j + 1))
                    pp = attps.tile([128, 512], F32, tag=f"at{m}", name="pp")
                    for k in range(2):
                        nc.tensor.matmul(pp[:], wsl(k, lo, lo + 128),
                                         out_all[k][:, js], start=(k == 0), stop=(k == 1))
                    nc.vector.tensor_add(
                        tok_upd[m][:, js], pp[:],
                        tok_new[m][:, bass.ds(dve_off, Q)][:, js])
            for m in range(2):
                lo = 1280 + 128 * m
                o_sb = normp.tile([128, Q], F32, tag="o_sb", name="o_sb")
                for j in range(2):
                    js = slice(512 * j, 512 * (j + 1))
                    pf = attps.tile([128, 512], F32, tag=f"at{m}", name="pf")
                    for k in range(2):
                        nc.tensor.matmul(pf[:], wsl(k, lo, lo + 128),
                                         tok_upd[k][:, js], start=(k == 0), stop=False)
                    nc.tensor.matmul(pf[:], bfeat_t[0:1, 128 * m:128 * (m + 1)],
                                     ones4k[:, js], start=False, stop=True)
                    nc.vector.scalar_tensor_tensor(
                        o_sb[:, js], pf[:], 0.2, xsl[m][:, js],
                        AluOp.mult, AluOp.add)
                nc.sync.dma_start(out_ext.ap()[128 * m:128 * (m + 1), :], o_sb[:])

            if dbg:
                for k in range(2):
                    r = slice(128 * k, 128 * (k + 1))
                    nc.sync.dma_start(dbg["d_token_new"].ap()[r, :], tok_new[k][:])
                    nc.sync.dma_start(dbg["d_out_all"].ap()[r, :], out_all[k][:])
                nc.sync.dma_start(dbg["d_kp0"].ap(), kp[0][:])
                nc.sync.dma_start(dbg["d_v1"].ap(),
                                  v1[:].rearrange("p c e -> p (c e)"))
                nc.sync.dma_start(dbg["d_q0"].ap(), qsb[0][:])


def pack_inputs(x, w_tok, b_tok, w_red, b_red, w_dil, b_dil, w_fuse, b_fuse,
                w_qkv, w_proj, w_feat, b_feat):
    """Host-side packing: full inputs -> list of 8 per-core input maps."""
    common = {}
    wcat = np.concatenate([
        np.ascontiguousarray(w_tok.T),
        np.ascontiguousarray(w_qkv[0:256].T),
        np.ascontiguousarray(w_qkv[256:512].T),
        np.ascontiguousarray(w_qkv[512:768].T),
        np.ascontiguousarray(w_proj.T),
        np.ascontiguousarray(w_feat),
        np.ascontiguousarray(w_red.T),
    ], axis=1).astype(BF)
    common["wcat"] = wcat

    # dil-conv lhsT: rows (K) = kh*42 + ci*3 + kw (per-kh im2col DMA order);
    # cols (M) = c*9 + j (feats row layout)
    w9 = [np.transpose(w_dil[d], (2, 1, 3, 0)).reshape(126, CMID)
          for d in range(3)]

    def mkL(blocks, bias):
        L = np.zeros((127, 126), np.float32)
        for j, (w, b) in enumerate(zip(blocks, bias)):
            if w is None:
                continue
            L[0:126, 14 * j:14 * (j + 1)] = w
            L[126, 14 * j:14 * (j + 1)] = b
        return L.astype(BF)

    L1a = mkL([w9[0]] * 6 + [None] * 3, [b_dil[0]] * 6 + [None] * 3)
    L1b = mkL([None] * 6 + [w9[1]] * 3, [None] * 6 + [b_dil[1]] * 3)
    L2a = mkL([w9[1]] * 3 + [None] * 6, [b_dil[1]] * 3 + [None] * 6)
    L2b = mkL([None] * 3 + [w9[2]] * 6, [None] * 3 + [b_dil[2]] * 6)
    common["Lcat"] = np.concatenate([L1a, L1b, L2a, L2b], axis=1)

    wf1 = np.ascontiguousarray(w_fuse[:, 0:126].T)
    wf2 = np.ascontiguousarray(w_fuse[:, 126:252].T)
    wf2 = np.vstack([wf2, np.asarray(b_fuse)[None, :]])
    wfusecat = np.zeros((127, 512), np.float32)
    wfusecat[:, 0:256] = wf2
    wfusecat[0:126, 256:512] = wf1
    common["wfusecat"] = wfusecat.astype(BF)

    bcat = np.zeros((128, 3), np.float32)
    bcat[:, 0] = b_tok[0:128]
    bcat[:, 1] = b_tok[128:256]
    bcat[0:CMID, 2] = b_red
    common["bcat"] = bcat
    common["bfeat_bf"] = np.asarray(b_feat).reshape(1, 256).astype(BF)

    in_maps = []
    for c in range(8):
        b, s = c // 4, c % 4
        m = dict(common)
        xb = x[b].reshape(256, N_TOK)
        m["xbf"] = np.ascontiguousarray(xb).astype(BF)
        m["xslice"] = np.ascontiguousarray(xb[:, s * Q:(s + 1) * Q], np.float32)
        m["qoff"] = np.array([[s * Q]], np.int32)
        in_maps.append(m)
    return in_maps


_NC_CACHE = {}


def get_nc(debug=False):
    if debug not in _NC_CACHE:
        _NC_CACHE[debug] = build(debug)
    return _NC_CACHE[debug]


def run(in_maps, debug=False, trace=False):
    nc = get_nc(debug)
    return run_bass_kernel_spmd(nc, in_maps, core_ids=list(range(8)), trace=trace)


def assemble(results, x):
    out = np.empty((2, 256, N_TOK), np.float32)
    for c, res in enumerate(results):
        b, s = c // 4, c % 4
        out[b, :, s * Q:(s + 1) * Q] = res["out"]
    return out.reshape(2, 256, 64, 64)


def kernel(**inputs):
    in_maps = pack_inputs(**inputs)
    r = run(in_maps)
    return assemble(r.results, inputs["x"])


# revision 22
# speedup vs baseline: 1.1140x; 1.1140x over previous
"""Trainium2 Bass kernel for nn_AMHSA (dense transformer block, B=2, C=256, 64x64).

Sharding: 8 cores, zero collectives. Core c handles batch b=c//4 and query
slice s=c%4 (1024 of 4096 spatial tokens). Each core in a batch group
redundantly computes the front (token embed + multi-scale dilated-conv bias +
K/V over all 4096 tokens), then attention for all 4 heads over its own
1024-query slice, projection, and the final residual for its slice. The
per-core query offset arrives as an int32 input applied through PE/DVE
register-offset (dynamic) access patterns, so all 8 cores run one SPMD
program.

All matmuls run in bf16 (fp32 PSUM accumulation). Weights are pre-packed,
pre-transposed, and concatenated host-side in pack_inputs(). The multi-scale
stack rows use a ci-major layout (row = c*9 + j) so the im2col copies and
rotation placements are single merged DMAs; the fuse weights are permuted to
match on the host. Softmax exp is split between the scalar engine (true exp)
and the vector engine (int16-bitcast bf16 fast exp, ~2-3% per-element, which
washes out over 4096-key softmax sums).
"""
import math
import numpy as np
import ml_dtypes
import concourse.bass as bass
import concourse.mybir as mybir
import concourse.tile as tile
from concourse import bacc
from concourse.bass_utils import run_bass_kernel_spmd

F32 = mybir.dt.float32
BF16 = mybir.dt.bfloat16
I32 = mybir.dt.int32
I16 = mybir.dt.int16
BF = ml_dtypes.bfloat16

NH, DH = 4, 64
CMID = 14
DILS = (1, 2, 3)
N_TOK = 4096
Q = 1024
NKC = N_TOK // 128
NJ = N_TOK // 512
AluOp = mybir.AluOpType
ActFn = mybir.ActivationFunctionType

# fast bf16 exp: bitcast(int16(round(x*FEXP_S + FEXP_B))) ~= exp(x*0.125)
FEXP_S = 0.125 * 128.0 / math.log(2.0)
FEXP_B = 127.0 * 128.0 - 5.6

# wcat column layout [256, 1550]:
# wtokT 0:256 | wqT 256:512 | wkT 512:768 | wvT 768:1024 | wprojT 1024:1280 |
# wfeat 1280:1536 | wredT 1536:1550


def build(debug=False):
    nc = bacc.Bacc("TRN2", target_bir_lowering=False, debug=False, num_devices=8)

    def param(name, shape, dt=BF16):
        return nc.declare_dram_parameter(name, list(shape), dt, isOutput=False)

    P = {}
    P["xbf"] = param("xbf", [256, N_TOK])
    P["xslice"] = param("xslice", [256, Q], F32)
    P["qoff"] = param("qoff", [1, 1], I32)
    P["wcat"] = param("wcat", [256, 1550])
    P["Lcat"] = param("Lcat", [127, 504])          # L1a|L1b|L2a|L2b
    P["wfusecat"] = param("wfusecat", [127, 512])  # [:,0:256]=wfuseT2, [0:126,256:512]=wfuseT1
    P["bcat"] = param("bcat", [128, 3], F32)       # btok0|btok1|bred(rows 0:14)
    P["bfeat_bf"] = param("bfeat_bf", [1, 256])
    P["out_ext"] = nc.declare_dram_parameter("out", [256, Q], F32, isOutput=True)
    dbg = {}
    if debug:
        for nm, shp, dt in [
            ("d_token_new", [256, N_TOK], BF16),
            ("d_kp0", [128, N_TOK], BF16),
            ("d_v1", [128, NKC * 4 * 65], BF16),
            ("d_q0", [128, Q], BF16),
            ("d_out_all", [256, Q], BF16),
        ]:
            dbg[nm] = nc.declare_dram_parameter(nm, shp, dt, isOutput=True)

    with tile.TileContext(nc) as tc:
        _emit(nc, tc, P, dbg)
    nc.compile()
    return nc


def _raw(t, offset, dims):
    return bass.AP(t[:].tensor, offset, [list(d) for d in dims])


def _emit(nc, tc, P, dbg):
    out_ext = P["out_ext"]

    import contextlib
    ctx = contextlib.ExitStack()
    with ctx:
        pers = ctx.enter_context(tc.tile_pool(name="pers", bufs=1))

        # ---------- persistent tiles ----------
        xsl = [pers.tile([128, Q], F32, tag=f"xsl{i}", name=f"xsl{i}") for i in range(2)]
        tok_new = [pers.tile([128, N_TOK], BF16, tag=f"tokn{i}", name=f"tokn{i}") for i in range(2)]
        kp = [pers.tile([128, N_TOK], BF16, tag=f"kp{i}", name=f"kp{i}") for i in range(2)]
        v1 = pers.tile([128, NKC, 4 * 65], BF16, tag="v1", name="v1")
        qsb = [pers.tile([128, Q], BF16, tag=f"q{i}", name=f"q{i}") for i in range(2)]
        out_all = [pers.tile([128, Q], BF16, tag=f"oa{i}", name=f"oa{i}") for i in range(2)]
        tok_upd = [pers.tile([128, Q], BF16, tag=f"tu{i}", name=f"tu{i}") for i in range(2)]
        qt = pers.tile([1, 1], I32, tag="qt", name="qt")
        ones4k = pers.tile([1, 4480], BF16, tag="ones4k", name="ones4k")

        wcat_t = [pers.tile([128, 1550], BF16, tag=f"wcat{k}", name=f"wcat{k}") for k in range(2)]
        Lcat_t = pers.tile([127, 504], BF16, tag="Lcat", name="Lcat")
        wfuse_t = pers.tile([127, 512], BF16, tag="wfuse", name="wfuse")
        bcat_t = pers.tile([128, 3], F32, tag="bcat", name="bcat")
        bfeat_t = pers.tile([1, 256], BF16, tag="bfeat", name="bfeat")

        def wsl(k, lo, hi):
            return wcat_t[k][:, lo:hi]

        for k in range(2):
            r = slice(128 * k, 128 * (k + 1))
            nc.sync.dma_start(wcat_t[k][:], P["wcat"].ap()[r, :])
            nc.sync.dma_start(xsl[k][:], P["xslice"].ap()[r, :])
        nc.sync.dma_start(Lcat_t[:], P["Lcat"].ap())
        nc.sync.dma_start(wfuse_t[:], P["wfusecat"].ap())
        nc.sync.dma_start(bcat_t[:], P["bcat"].ap())
        nc.sync.dma_start(bfeat_t[:], P["bfeat_bf"].ap())
        nc.sync.dma_start(qt[:], P["qoff"].ap())
        nc.gpsimd.memset(ones4k[:], 1.0)
        v1h = v1[:].rearrange("p c (h e) -> p c h e", e=65)
        nc.gpsimd.memset(v1h[:, :, :, 64:65], 1.0)

        btok_t = [bcat_t[:, k:k + 1] for k in range(2)]
        bred_t = bcat_t[0:CMID, 2:3]

        # per-core query offset registers
        pe_reg = nc.alloc_register(mybir.EngineType.PE, "qoff_pe")
        dve_reg = nc.alloc_register(mybir.EngineType.DVE, "qoff_dve")
        nc.tensor.reg_load(pe_reg, qt[0:1, 0:1])
        nc.vector.reg_load(dve_reg, qt[0:1, 0:1])
        pe_off = nc.snap(pe_reg, min_val=0, max_val=N_TOK - Q, donate=True)
        dve_off = nc.snap(dve_reg, min_val=0, max_val=N_TOK - Q, donate=True)

        with (
            tc.tile_pool(name="front", bufs=1) as front,
            tc.tile_pool(name="psA", bufs=6, space="PSUM") as psA,
        ):
            tok_bf = [front.tile([128, N_TOK], BF16, tag=f"tokbf{i}", name=f"tokbf{i}") for i in range(2)]
            z_pad = front.tile([CMID, 72, 70], BF16, tag="z_pad", name="z_pad")
            rotz = [front.tile([126, N_TOK], BF16, tag=f"rotz{i}", name=f"rotz{i}") for i in range(2)]
            feats1 = front.tile([126, N_TOK], BF16, tag="feats1", name="feats1")
            feats2 = front.tile([127, N_TOK], BF16, tag="feats2", name="feats2")

            # ---- stage A: token = wtok @ x + btok (copies on ACT) ----
            with tc.tile_pool(name="xpool", bufs=1) as xpool:
                xbf = [xpool.tile([128, N_TOK], BF16, tag=f"xbf{i}", name=f"xbf{i}") for i in range(2)]
                for k in range(2):
                    for cq in range(2):
                        cs = slice(2048 * cq, 2048 * (cq + 1))
                        nc.sync.dma_start(xbf[k][:, cs],
                                          P["xbf"].ap()[128 * k:128 * (k + 1), cs])
                for j in range(NJ):
                    js = slice(512 * j, 512 * (j + 1))
                    for m in range(2):
                        pt = psA.tile([128, 512], F32, tag="ps", name="ps")
                        for k in range(2):
                            nc.tensor.matmul(pt[:], wsl(k, 128 * m, 128 * (m + 1)),
                                             xbf[k][:, js], start=(k == 0), stop=(k == 1))
                        if m == 0:
                            nc.scalar.activation(tok_bf[m][:, js], pt[:],
                                                 ActFn.Identity, bias=btok_t[m])
                        else:
                            nc.vector.tensor_scalar_add(tok_bf[m][:, js], pt[:],
                                                        btok_t[m])

            # ---- stage B: z (zero-padded) + rotations ----
            nc.gpsimd.memset(z_pad[:], 0.0)
            for j in range(NJ):
                js = slice(512 * j, 512 * (j + 1))
                pz = psA.tile([CMID, 512], F32, tag="ps", name="ps")
                for k in range(2):
                    nc.tensor.matmul(pz[:], wsl(k, 1536, 1550), tok_bf[k][:, js],
                                     start=(k == 0), stop=(k == 1))
                zdst = z_pad[:, 4 + 8 * j:4 + 8 * (j + 1), 3:67]
                zsrc = pz[:].rearrange("c (a b) -> c a b", b=64)
                if j % 2 == 0:
                    nc.scalar.activation(zdst, zsrc, ActFn.Identity, bias=bred_t)
                else:
                    nc.vector.tensor_scalar_add(zdst, zsrc, bred_t)

            z3 = z_pad[:, 4:68, 3:67]
            rots = [
                z3,                                      # r0
                z3[:, :, ::-1],                          # r1 flip W
                z3[:, ::-1, :],                          # r2 flip H
                z3[:, :, ::-1].transpose([0, 2, 1]),     # r3 rot90
                z3[:, ::-1, ::-1],                       # r4 rot180
                z3.transpose([0, 2, 1])[:, :, ::-1],     # r5 rot270
            ]
            # rotz rows are j-major (row = j*14 + c): plain block placements
            for r in range(6):
                rt = front.tile([CMID, 64, 64], BF16, tag="rt",
                                name=f"rt{r}", bufs=2)
                if r % 2 == 0:
                    nc.vector.tensor_copy(rt[:], rots[r])
                else:
                    nc.scalar.copy(rt[:], rots[r])
                nc.sync.dma_start(rotz[0][14 * r:14 * (r + 1), :],
                                  rt[:].rearrange("c h w -> c (h w)"))
            for cq in range(4):
                cs = slice(1024 * cq, 1024 * (cq + 1))
                nc.sync.dma_start(rotz[0][84:126, cs], rotz[0][0:42, cs])
                nc.sync.dma_start(rotz[1][0:42, cs], rotz[0][42:84, cs])
                nc.sync.dma_start(rotz[1][42:126, cs], rotz[0][0:84, cs])

            # ---- stages C+D: im2col (one merged DMA each) + convs + feats ----
            ZPP = 72 * 70

            def build_z9(di):
                d = DILS[di]
                t = front.tile([127, 64, 70], BF16, tag="z9",
                               name=f"z9_{di}", bufs=2)
                tf = t[:].rearrange("k h w -> k (h w)")
                for kh in range(3):
                    srcp = _raw(z_pad, (4 + d * (kh - 1)) * 70 - d,
                                [[ZPP, CMID], [d, 3], [1, 4480]])
                    dstp = _raw(t, kh * 42 * 4480, [[4480, 42], [1, 4480]])
                    nc.sync.dma_start(dstp, srcp)
                nc.sync.dma_start(tf[126:127, :], ones4k[:])
                return t

            def emit_stack(fdst, la, lb, za, zb, rz):
                for j in range(NJ):
                    js = slice(512 * j, 512 * (j + 1))
                    py = psA.tile([126, 512], F32, tag="ps", name="py")
                    nc.tensor.matmul(py[:], Lcat_t[:, 126 * la:126 * (la + 1)],
                                     za[:][:, 8 * j:8 * (j + 1), 3:67],
                                     start=True, stop=False)
                    nc.tensor.matmul(py[:], Lcat_t[:, 126 * lb:126 * (lb + 1)],
                                     zb[:][:, 8 * j:8 * (j + 1), 3:67],
                                     start=False, stop=True)
                    nc.vector.tensor_mul(fdst[0:126, js], py[:], rz[:, js])

            z9_0 = build_z9(0)
            z9_1 = build_z9(1)
            emit_stack(feats1, 0, 1, z9_0, z9_1, rotz[0])
            z9_2 = build_z9(2)
            emit_stack(feats2, 2, 3, z9_1, z9_2, rotz[1])
            nc.sync.dma_start(feats2[126:127, :], ones4k[:, 0:N_TOK])

            # ---- stage E: fuse conv + residual -> token_new ----
            for m in range(2):
                ms = slice(128 * m, 128 * (m + 1))
                ms2 = slice(256 + 128 * m, 256 + 128 * (m + 1))
                for j in range(NJ):
                    js = slice(512 * j, 512 * (j + 1))
                    pf = psA.tile([128, 512], F32, tag="ps", name="ps")
                    nc.tensor.matmul(pf[:], wfuse_t[0:126, ms2], feats1[:, js],
                                     start=True, stop=False)
                    nc.tensor.matmul(pf[:], wfuse_t[:, ms], feats2[:, js],
                                     start=False, stop=True)
                    nc.vector.tensor_add(tok_new[m][:, js], pf[:], tok_bf[m][:, js])

            # ---- stage F: K pairs, V^T, Q ----
            for p in range(2):
                lo = 512 + 128 * p
                for j in range(NJ):
                    js = slice(512 * j, 512 * (j + 1))
                    pk = psA.tile([128, 512], F32, tag="ps", name="ps")
                    for k in range(2):
                        nc.tensor.matmul(pk[:], wsl(k, lo, lo + 128),
                                         tok_new[k][:, js], start=(k == 0), stop=(k == 1))
                    if p == 0:
                        nc.scalar.copy(kp[p][:, js], pk[:])
                    else:
                        nc.vector.tensor_copy(kp[p][:, js], pk[:])
            for t in range(NKC):
                ts_ = slice(128 * t, 128 * (t + 1))
                pv = psA.tile([128, 256], F32, tag="ps", name="ps")
                for k in range(2):
                    nc.tensor.matmul(pv[:], tok_new[k][:, ts_], wsl(k, 768, 1024),
                                     start=(k == 0), stop=(k == 1))
                pv4 = pv[:].rearrange("p (h e) -> p h e", e=64)
                nc.vector.tensor_copy(v1h[:, t, :, 0:64], pv4)
            for p in range(2):
                lo = 256 + 128 * p
                for j in range(2):
                    pq = psA.tile([128, 512], F32, tag="ps", name="ps")
                    for k in range(2):
                        rhs = tok_new[k][:, bass.ds(pe_off, Q)][:, 512 * j:512 * (j + 1)]
                        nc.tensor.matmul(pq[:], wsl(k, lo, lo + 128), rhs,
                                         start=(k == 0), stop=(k == 1))
                    nc.scalar.copy(qsb[p][:, 512 * j:512 * (j + 1)], pq[:])

        # ---------- attention ----------
        # Two passes per head pair, one per 512-query half: av PSUM shrinks to
        # one bank per head, freeing room for 3-deep att PSUM buffering so the
        # PE streams without gating on the exp engines.
        with (
            tc.tile_pool(name="attps", bufs=3, space="PSUM") as attps,
            tc.tile_pool(name="avps", bufs=1, space="PSUM") as avps,
            tc.tile_pool(name="expp", bufs=4) as expp,
            tc.tile_pool(name="normp", bufs=1) as normp,
        ):
            for p in range(2):
                for j in range(2):
                    js = slice(512 * j, 512 * (j + 1))
                    av = [avps.tile([65, 512], F32, tag=f"av{h}", name=f"av{h}")
                          for h in range(2)]
                    for kc in range(NKC):
                        kcs = slice(128 * kc, 128 * (kc + 1))
                        at = [None, None]
                        for h in range(2):
                            hr = slice(64 * h, 64 * (h + 1))
                            a = attps.tile([128, 512], F32, tag=f"at{h}", name=f"at{h}")
                            nc.tensor.matmul(a[:], kp[p][hr, kcs],
                                             qsb[p][hr, js], start=True, stop=True)
                            at[h] = a
                        e0 = expp.tile([128, 512], BF16, tag="e0", name="e0")
                        nc.scalar.activation(e0[:], at[0][:], ActFn.Exp, scale=0.125)
                        e1 = expp.tile([128, 512], I16, tag="e1", name="e1")
                        nc.vector.tensor_scalar(e1[:], at[1][:], FEXP_S, FEXP_B,
                                                AluOp.mult, AluOp.add)
                        ex = [e0[:], e1[:].bitcast(BF16)]
                        for h in range(2):
                            head = 2 * p + h
                            nc.tensor.matmul(
                                av[h][:], v1h[:, kc, head, :], ex[h],
                                start=(kc == 0), stop=(kc == NKC - 1))
                    # normalize this query half
                    for h in range(2):
                        r_f = normp.tile([1, 512], F32, tag="r_f", name="r_f")
                        r_bf = normp.tile([1, 512], BF16, tag="r_bf", name="r_bf")
                        bc_sb = normp.tile([64, 512], BF16, tag="bc_sb", name="bc_sb")
                        nc.vector.reciprocal(r_f[:], av[h][64:65, :])
                        nc.vector.tensor_copy(r_bf[:], r_f[:])
                        bc = attps.tile([64, 512], F32, tag=f"at{h}", name="bc")
                        nc.tensor.matmul(bc[:], ones4k[:, 0:64], r_bf[:],
                                         start=True, stop=True)
                        nc.scalar.copy(bc_sb[:], bc[:])
                        nc.vector.tensor_mul(
                            out_all[p][64 * h:64 * (h + 1), js],
                            av[h][0:64, :], bc_sb[:])

            # ---------- tail: proj + residual + feat + output ----------
            for m in range(2):
                lo = 1024 + 128 * m
                for j in range(2):
                    js = slice(512 * j, 512 * (j + 1))
                    pp = attps.tile([128, 512], F32, tag=f"at{m}", name="pp")
                    for k in range(2):
                        nc.tensor.matmul(pp[:], wsl(k, lo, lo + 128),
                                         out_all[k][:, js], start=(k == 0), stop=(k == 1))
                    nc.vector.tensor_add(
                        tok_upd[m][:, js], pp[:],
                        tok_new[m][:, bass.ds(dve_off, Q)][:, js])
            for m in range(2):
                lo = 1280 + 128 * m
                o_sb = normp.tile([128, Q], F32, tag="o_sb", name="o_sb")
                for j in range(2):
                    js = slice(512 * j, 512 * (j + 1))
                    pf = attps.tile([128, 512], F32, tag=f"at{m}", name="pf")
                    for k in range(2):
                        nc.tensor.matmul(pf[:], wsl(k, lo, lo + 128),
                                         tok_upd[k][:, js], start=(k == 0), stop=False)
                    nc.tensor.matmul(pf[:], bfeat_t[0:1, 128 * m:128 * (m + 1)],
                                     ones4k[:, js], start=False, stop=True)
                    nc.vector.scalar_tensor_tensor(
                        o_sb[:, js], pf[:], 0.2, xsl[m][:, js],
                        AluOp.mult, AluOp.add)
                nc.sync.dma_start(out_ext.ap()[128 * m:128 * (m + 1), :], o_sb[:])

            if dbg:
                for k in range(2):
                    r = slice(128 * k, 128 * (k + 1))
                    nc.sync.dma_start(dbg["d_token_new"].ap()[r, :], tok_new[k][:])
                    nc.sync.dma_start(dbg["d_out_all"].ap()[r, :], out_all[k][:])
                nc.sync.dma_start(dbg["d_kp0"].ap(), kp[0][:])
                nc.sync.dma_start(dbg["d_v1"].ap(),
                                  v1[:].rearrange("p c e -> p (c e)"))
                nc.sync.dma_start(dbg["d_q0"].ap(), qsb[0][:])


def pack_inputs(x, w_tok, b_tok, w_red, b_red, w_dil, b_dil, w_fuse, b_fuse,
                w_qkv, w_proj, w_feat, b_feat):
    """Host-side packing: full inputs -> list of 8 per-core input maps."""
    common = {}
    wcat = np.concatenate([
        np.ascontiguousarray(w_tok.T),
        np.ascontiguousarray(w_qkv[0:256].T),
        np.ascontiguousarray(w_qkv[256:512].T),
        np.ascontiguousarray(w_qkv[512:768].T),
        np.ascontiguousarray(w_proj.T),
        np.ascontiguousarray(w_feat),
        np.ascontiguousarray(w_red.T),
    ], axis=1).astype(BF)
    common["wcat"] = wcat

    # dil-conv lhsT: rows (K) = kh*42 + ci*3 + kw (per-kh im2col DMA order);
    # cols (M) = c*9 + j (feats row layout)
    w9 = [np.transpose(w_dil[d], (2, 1, 3, 0)).reshape(126, CMID)
          for d in range(3)]

    def mkL(blocks, bias):
        L = np.zeros((127, 126), np.float32)
        for j, (w, b) in enumerate(zip(blocks, bias)):
            if w is None:
                continue
            L[0:126, 14 * j:14 * (j + 1)] = w
            L[126, 14 * j:14 * (j + 1)] = b
        return L.astype(BF)

    L1a = mkL([w9[0]] * 6 + [None] * 3, [b_dil[0]] * 6 + [None] * 3)
    L1b = mkL([None] * 6 + [w9[1]] * 3, [None] * 6 + [b_dil[1]] * 3)
    L2a = mkL([w9[1]] * 3 + [None] * 6, [b_dil[1]] * 3 + [None] * 6)
    L2b = mkL([None] * 3 + [w9[2]] * 6, [None] * 3 + [b_dil[2]] * 6)
    common["Lcat"] = np.concatenate([L1a, L1b, L2a, L2b], axis=1)

    wf1 = np.ascontiguousarray(w_fuse[:, 0:126].T)
    wf2 = np.ascontiguousarray(w_fuse[:, 126:252].T)
    wf2 = np.vstack([wf2, np.asarray(b_fuse)[None, :]])
    wfusecat = np.zeros((127, 512), np.float32)
    wfusecat[:, 0:256] = wf2
    wfusecat[0:126, 256:512] = wf1
    common["wfusecat"] = wfusecat.astype(BF)

    bcat = np.zeros((128, 3), np.float32)
    bcat[:, 0] = b_tok[0:128]
    bcat[:, 1] = b_tok[128:256]
    bcat[0:CMID, 2] = b_red
    common["bcat"] = bcat
    common["bfeat_bf"] = np.asarray(b_feat).reshape(1, 256).astype(BF)

    in_maps = []
    for c in range(8):
        b, s = c // 4, c % 4
        m = dict(common)
        xb = x[b].reshape(256, N_TOK)
        m["xbf"] = np.ascontiguousarray(xb).astype(BF)
        m["xslice"] = np.ascontiguousarray(xb[:, s * Q:(s + 1) * Q], np.float32)
        m["qoff"] = np.array([[s * Q]], np.int32)
        in_maps.append(m)
    return in_maps


_NC_CACHE = {}


def get_nc(debug=False):
    if debug not in _NC_CACHE:
        _NC_CACHE[debug] = build(debug)
    return _NC_CACHE[debug]


def run(in_maps, debug=False, trace=False):
    nc = get_nc(debug)
    return run_bass_kernel_spmd(nc, in_maps, core_ids=list(range(8)), trace=trace)


def assemble(results, x):
    out = np.empty((2, 256, N_TOK), np.float32)
    for c, res in enumerate(results):
        b, s = c // 4, c % 4
        out[b, :, s * Q:(s + 1) * Q] = res["out"]
    return out.reshape(2, 256, 64, 64)


def kernel(**inputs):
    in_maps = pack_inputs(**inputs)
    r = run(in_maps)
    return assemble(r.results, inputs["x"])


# revision 23
# speedup vs baseline: 1.2146x; 1.0903x over previous
"""Trainium2 Bass kernel for nn_AMHSA (dense transformer block, B=2, C=256, 64x64).

Sharding: 8 cores, zero collectives. Core c handles batch b=c//4 and query
slice s=c%4 (1024 of 4096 spatial tokens). Each core in a batch group
redundantly computes the front (token embed + multi-scale dilated-conv bias +
K/V over all 4096 tokens), then attention for all 4 heads over its own
1024-query slice, projection, and the final residual for its slice. The
per-core query offset arrives as an int32 input applied through PE/DVE
register-offset (dynamic) access patterns, so all 8 cores run one SPMD
program.

All matmuls run in bf16 (fp32 PSUM accumulation). Weights are pre-packed,
pre-transposed, and concatenated host-side in pack_inputs(). The multi-scale
stack rows use a ci-major layout (row = c*9 + j) so the im2col copies and
rotation placements are single merged DMAs; the fuse weights are permuted to
match on the host. Softmax exp is split between the scalar engine (true exp)
and the vector engine (int16-bitcast bf16 fast exp, ~2-3% per-element, which
washes out over 4096-key softmax sums).
"""
import math
import numpy as np
import ml_dtypes
import concourse.bass as bass
import concourse.mybir as mybir
import concourse.tile as tile
from concourse import bacc
from concourse.bass_utils import run_bass_kernel_spmd

F32 = mybir.dt.float32
BF16 = mybir.dt.bfloat16
I32 = mybir.dt.int32
I16 = mybir.dt.int16
BF = ml_dtypes.bfloat16

NH, DH = 4, 64
CMID = 14
DILS = (1, 2, 3)
N_TOK = 4096
Q = 1024
NKC = N_TOK // 128
NJ = N_TOK // 512
AluOp = mybir.AluOpType
ActFn = mybir.ActivationFunctionType

# fast bf16 exp: bitcast(int16(round(x*FEXP_S + FEXP_B))) ~= exp(x*0.125)
FEXP_S = 0.125 * 128.0 / math.log(2.0)
FEXP_B = 127.0 * 128.0 - 5.6

# wcat column layout [256, 1550]:
# wtokT 0:256 | wqT 256:512 | wkT 512:768 | wvT 768:1024 | wprojT 1024:1280 |
# wfeat 1280:1536 | wredT 1536:1550


def build(debug=False):
    nc = bacc.Bacc("TRN2", target_bir_lowering=False, debug=False, num_devices=8)

    def param(name, shape, dt=BF16):
        return nc.declare_dram_parameter(name, list(shape), dt, isOutput=False)

    P = {}
    P["xbf"] = param("xbf", [256, N_TOK])
    P["xslice"] = param("xslice", [256, Q], F32)
    P["qoff"] = param("qoff", [1, 1], I32)
    P["wcat"] = param("wcat", [256, 1550])
    P["Lcat"] = param("Lcat", [127, 504])          # L1a|L1b|L2a|L2b
    P["wfusecat"] = param("wfusecat", [127, 512])  # [:,0:256]=wfuseT2, [0:126,256:512]=wfuseT1
    P["bcat"] = param("bcat", [128, 3], F32)       # btok0|btok1|bred(rows 0:14)
    P["bfeat_bf"] = param("bfeat_bf", [1, 256])
    P["out_ext"] = nc.declare_dram_parameter("out", [256, Q], F32, isOutput=True)
    dbg = {}
    if debug:
        for nm, shp, dt in [
            ("d_token_new", [256, N_TOK], BF16),
            ("d_kp0", [128, N_TOK], BF16),
            ("d_v1", [128, NKC * 4 * 65], BF16),
            ("d_q0", [128, Q], BF16),
            ("d_out_all", [256, Q], BF16),
        ]:
            dbg[nm] = nc.declare_dram_parameter(nm, shp, dt, isOutput=True)

    with tile.TileContext(nc) as tc:
        _emit(nc, tc, P, dbg)
    nc.compile()
    return nc


def _raw(t, offset, dims):
    return bass.AP(t[:].tensor, offset, [list(d) for d in dims])


def _emit(nc, tc, P, dbg):
    out_ext = P["out_ext"]

    import contextlib
    ctx = contextlib.ExitStack()
    with ctx:
        pers = ctx.enter_context(tc.tile_pool(name="pers", bufs=1))

        # ---------- persistent tiles ----------
        xsl = [pers.tile([128, Q], F32, tag=f"xsl{i}", name=f"xsl{i}") for i in range(2)]
        tok_new = [pers.tile([128, N_TOK], BF16, tag=f"tokn{i}", name=f"tokn{i}") for i in range(2)]
        kp = [pers.tile([128, N_TOK], BF16, tag=f"kp{i}", name=f"kp{i}") for i in range(2)]
        v1 = pers.tile([128, NKC, 4 * 65], BF16, tag="v1", name="v1")
        qsb = [pers.tile([128, Q], BF16, tag=f"q{i}", name=f"q{i}") for i in range(2)]
        out_all = [pers.tile([128, Q], BF16, tag=f"oa{i}", name=f"oa{i}") for i in range(2)]
        tok_upd = [pers.tile([128, Q], BF16, tag=f"tu{i}", name=f"tu{i}") for i in range(2)]
        qt = pers.tile([1, 1], I32, tag="qt", name="qt")
        ones4k = pers.tile([1, 4480], BF16, tag="ones4k", name="ones4k")

        wcat_t = [pers.tile([128, 1550], BF16, tag=f"wcat{k}", name=f"wcat{k}") for k in range(2)]
        Lcat_t = pers.tile([127, 504], BF16, tag="Lcat", name="Lcat")
        wfuse_t = pers.tile([127, 512], BF16, tag="wfuse", name="wfuse")
        bcat_t = pers.tile([128, 3], F32, tag="bcat", name="bcat")
        bfeat_t = pers.tile([1, 256], BF16, tag="bfeat", name="bfeat")

        def wsl(k, lo, hi):
            return wcat_t[k][:, lo:hi]

        for k in range(2):
            r = slice(128 * k, 128 * (k + 1))
            nc.sync.dma_start(wcat_t[k][:], P["wcat"].ap()[r, :])
        nc.sync.dma_start(bcat_t[:], P["bcat"].ap())
        nc.sync.dma_start(qt[:], P["qoff"].ap())
        nc.gpsimd.memset(ones4k[:], 1.0)
        v1h = v1[:].rearrange("p c (h e) -> p c h e", e=65)
        nc.gpsimd.memset(v1h[:, :, :, 64:65], 1.0)

        btok_t = [bcat_t[:, k:k + 1] for k in range(2)]
        bred_t = bcat_t[0:CMID, 2:3]

        # per-core query offset registers
        pe_reg = nc.alloc_register(mybir.EngineType.PE, "qoff_pe")
        dve_reg = nc.alloc_register(mybir.EngineType.DVE, "qoff_dve")
        nc.tensor.reg_load(pe_reg, qt[0:1, 0:1])
        nc.vector.reg_load(dve_reg, qt[0:1, 0:1])
        pe_off = nc.snap(pe_reg, min_val=0, max_val=N_TOK - Q, donate=True)
        dve_off = nc.snap(dve_reg, min_val=0, max_val=N_TOK - Q, donate=True)

        with (
            tc.tile_pool(name="front", bufs=1) as front,
            tc.tile_pool(name="psA", bufs=6, space="PSUM") as psA,
        ):
            tok_bf = [front.tile([128, N_TOK], BF16, tag=f"tokbf{i}", name=f"tokbf{i}") for i in range(2)]
            z_pad = front.tile([CMID, 72, 70], BF16, tag="z_pad", name="z_pad")
            rotz = [front.tile([126, N_TOK], BF16, tag=f"rotz{i}", name=f"rotz{i}") for i in range(2)]
            feats1 = front.tile([126, N_TOK], BF16, tag="feats1", name="feats1")
            feats2 = front.tile([127, N_TOK], BF16, tag="feats2", name="feats2")

            # ---- stage A: token = wtok @ x + btok (copies on ACT) ----
            with tc.tile_pool(name="xpool", bufs=1) as xpool:
                xbf = [xpool.tile([128, N_TOK], BF16, tag=f"xbf{i}", name=f"xbf{i}") for i in range(2)]
                for k in range(2):
                    for cq in range(2):
                        cs = slice(2048 * cq, 2048 * (cq + 1))
                        nc.sync.dma_start(xbf[k][:, cs],
                                          P["xbf"].ap()[128 * k:128 * (k + 1), cs])
                for j in range(NJ):
                    js = slice(512 * j, 512 * (j + 1))
                    for m in range(2):
                        pt = psA.tile([128, 512], F32, tag="ps", name="ps")
                        for k in range(2):
                            nc.tensor.matmul(pt[:], wsl(k, 128 * m, 128 * (m + 1)),
                                             xbf[k][:, js], start=(k == 0), stop=(k == 1))
                        if m == 0:
                            nc.scalar.activation(tok_bf[m][:, js], pt[:],
                                                 ActFn.Identity, bias=btok_t[m])
                        else:
                            nc.vector.tensor_scalar_add(tok_bf[m][:, js], pt[:],
                                                        btok_t[m])

            # deferred input loads (needed from stage C onward)
            nc.gpsimd.dma_start(Lcat_t[:], P["Lcat"].ap())
            nc.gpsimd.dma_start(wfuse_t[:], P["wfusecat"].ap())
            nc.gpsimd.dma_start(bfeat_t[:], P["bfeat_bf"].ap())
            for k in range(2):
                nc.gpsimd.dma_start(xsl[k][:],
                                    P["xslice"].ap()[128 * k:128 * (k + 1), :])

            # ---- stage B: z (zero-padded) + rotations ----
            nc.gpsimd.memset(z_pad[:], 0.0)
            for j in range(NJ):
                js = slice(512 * j, 512 * (j + 1))
                pz = psA.tile([CMID, 512], F32, tag="ps", name="ps")
                for k in range(2):
                    nc.tensor.matmul(pz[:], wsl(k, 1536, 1550), tok_bf[k][:, js],
                                     start=(k == 0), stop=(k == 1))
                zdst = z_pad[:, 4 + 8 * j:4 + 8 * (j + 1), 3:67]
                zsrc = pz[:].rearrange("c (a b) -> c a b", b=64)
                if j % 2 == 0:
                    nc.scalar.activation(zdst, zsrc, ActFn.Identity, bias=bred_t)
                else:
                    nc.vector.tensor_scalar_add(zdst, zsrc, bred_t)

            z3 = z_pad[:, 4:68, 3:67]
            rots = [
                z3,                                      # r0
                z3[:, :, ::-1],                          # r1 flip W
                z3[:, ::-1, :],                          # r2 flip H
                z3[:, :, ::-1].transpose([0, 2, 1]),     # r3 rot90
                z3[:, ::-1, ::-1],                       # r4 rot180
                z3.transpose([0, 2, 1])[:, :, ::-1],     # r5 rot270
            ]
            # rotz rows are j-major (row = j*14 + c): plain block placements
            for r in range(6):
                rt = front.tile([CMID, 64, 64], BF16, tag="rt",
                                name=f"rt{r}", bufs=2)
                if r % 2 == 0:
                    nc.vector.tensor_copy(rt[:], rots[r])
                else:
                    nc.scalar.copy(rt[:], rots[r])
                nc.sync.dma_start(rotz[0][14 * r:14 * (r + 1), :],
                                  rt[:].rearrange("c h w -> c (h w)"))
            for cq in range(4):
                cs = slice(1024 * cq, 1024 * (cq + 1))
                nc.sync.dma_start(rotz[0][84:126, cs], rotz[0][0:42, cs])
                nc.sync.dma_start(rotz[1][0:42, cs], rotz[0][42:84, cs])
                nc.sync.dma_start(rotz[1][42:126, cs], rotz[0][0:84, cs])

            # ---- stages C+D: im2col (one merged DMA each) + convs + feats ----
            ZPP = 72 * 70

            def build_z9(di):
                d = DILS[di]
                t = front.tile([127, 64, 70], BF16, tag="z9",
                               name=f"z9_{di}", bufs=2)
                tf = t[:].rearrange("k h w -> k (h w)")
                for kh in range(3):
                    srcp = _raw(z_pad, (4 + d * (kh - 1)) * 70 - d,
                                [[ZPP, CMID], [d, 3], [1, 4480]])
                    dstp = _raw(t, kh * 42 * 4480, [[4480, 42], [1, 4480]])
                    nc.gpsimd.dma_start(dstp, srcp)
                nc.gpsimd.dma_start(tf[126:127, :], ones4k[:])
                return t

            def emit_stack(fdst, la, lb, za, zb, rz):
                for j in range(NJ):
                    js = slice(512 * j, 512 * (j + 1))
                    py = psA.tile([126, 512], F32, tag="ps", name="py")
                    nc.tensor.matmul(py[:], Lcat_t[:, 126 * la:126 * (la + 1)],
                                     za[:][:, 8 * j:8 * (j + 1), 3:67],
                                     start=True, stop=False)
                    nc.tensor.matmul(py[:], Lcat_t[:, 126 * lb:126 * (lb + 1)],
                                     zb[:][:, 8 * j:8 * (j + 1), 3:67],
                                     start=False, stop=True)
                    nc.vector.tensor_mul(fdst[0:126, js], py[:], rz[:, js])

            z9_0 = build_z9(0)
            z9_1 = build_z9(1)
            emit_stack(feats1, 0, 1, z9_0, z9_1, rotz[0])
            z9_2 = build_z9(2)
            emit_stack(feats2, 2, 3, z9_1, z9_2, rotz[1])
            nc.sync.dma_start(feats2[126:127, :], ones4k[:, 0:N_TOK])

            # ---- stage E: fuse conv + residual -> token_new ----
            for m in range(2):
                ms = slice(128 * m, 128 * (m + 1))
                ms2 = slice(256 + 128 * m, 256 + 128 * (m + 1))
                for j in range(NJ):
                    js = slice(512 * j, 512 * (j + 1))
                    pf = psA.tile([128, 512], F32, tag="ps", name="ps")
                    nc.tensor.matmul(pf[:], wfuse_t[0:126, ms2], feats1[:, js],
                                     start=True, stop=False)
                    nc.tensor.matmul(pf[:], wfuse_t[:, ms], feats2[:, js],
                                     start=False, stop=True)
                    nc.vector.tensor_add(tok_new[m][:, js], pf[:], tok_bf[m][:, js])

            # ---- stage F: K pairs, V^T, Q ----
            for p in range(2):
                lo = 512 + 128 * p
                for j in range(NJ):
                    js = slice(512 * j, 512 * (j + 1))
                    pk = psA.tile([128, 512], F32, tag="ps", name="ps")
                    for k in range(2):
                        nc.tensor.matmul(pk[:], wsl(k, lo, lo + 128),
                                         tok_new[k][:, js], start=(k == 0), stop=(k == 1))
                    if p == 0:
                        nc.scalar.copy(kp[p][:, js], pk[:])
                    else:
                        nc.vector.tensor_copy(kp[p][:, js], pk[:])
            for t in range(NKC):
                ts_ = slice(128 * t, 128 * (t + 1))
                pv = psA.tile([128, 256], F32, tag="ps", name="ps")
                for k in range(2):
                    nc.tensor.matmul(pv[:], tok_new[k][:, ts_], wsl(k, 768, 1024),
                                     start=(k == 0), stop=(k == 1))
                pv4 = pv[:].rearrange("p (h e) -> p h e", e=64)
                nc.vector.tensor_copy(v1h[:, t, :, 0:64], pv4)
            for p in range(2):
                lo = 256 + 128 * p
                for j in range(2):
                    pq = psA.tile([128, 512], F32, tag="ps", name="ps")
                    for k in range(2):
                        rhs = tok_new[k][:, bass.ds(pe_off, Q)][:, 512 * j:512 * (j + 1)]
                        nc.tensor.matmul(pq[:], wsl(k, lo, lo + 128), rhs,
                                         start=(k == 0), stop=(k == 1))
                    nc.scalar.copy(qsb[p][:, 512 * j:512 * (j + 1)], pq[:])

        # ---------- attention ----------
        # Two passes per head pair, one per 512-query half: av PSUM shrinks to
        # one bank per head, freeing room for 3-deep att PSUM buffering so the
        # PE streams without gating on the exp engines.
        with (
            tc.tile_pool(name="attps", bufs=3, space="PSUM") as attps,
            tc.tile_pool(name="avps", bufs=1, space="PSUM") as avps,
            tc.tile_pool(name="expp", bufs=4) as expp,
            tc.tile_pool(name="normp", bufs=1) as normp,
        ):
            for p in range(2):
                for j in range(2):
                    js = slice(512 * j, 512 * (j + 1))
                    av = [avps.tile([65, 512], F32, tag=f"av{h}", name=f"av{h}")
                          for h in range(2)]
                    for kc in range(NKC):
                        kcs = slice(128 * kc, 128 * (kc + 1))
                        at = [None, None]
                        for h in range(2):
                            hr = slice(64 * h, 64 * (h + 1))
                            a = attps.tile([128, 512], F32, tag=f"at{h}", name=f"at{h}")
                            nc.tensor.matmul(a[:], kp[p][hr, kcs],
                                             qsb[p][hr, js], start=True, stop=True)
                            at[h] = a
                        e0 = expp.tile([128, 512], BF16, tag="e0", name="e0")
                        nc.scalar.activation(e0[:], at[0][:], ActFn.Exp, scale=0.125)
                        e1 = expp.tile([128, 512], I16, tag="e1", name="e1")
                        nc.vector.tensor_scalar(e1[:], at[1][:], FEXP_S, FEXP_B,
                                                AluOp.mult, AluOp.add)
                        ex = [e0[:], e1[:].bitcast(BF16)]
                        for h in range(2):
                            head = 2 * p + h
                            nc.tensor.matmul(
                                av[h][:], v1h[:, kc, head, :], ex[h],
                                start=(kc == 0), stop=(kc == NKC - 1))
                    # normalize this query half
                    for h in range(2):
                        r_f = normp.tile([1, 512], F32, tag="r_f", name="r_f")
                        r_bf = normp.tile([1, 512], BF16, tag="r_bf", name="r_bf")
                        bc_sb = normp.tile([64, 512], BF16, tag="bc_sb", name="bc_sb")
                        nc.vector.reciprocal(r_f[:], av[h][64:65, :])
                        nc.vector.tensor_copy(r_bf[:], r_f[:])
                        bc = attps.tile([64, 512], F32, tag=f"at{h}", name="bc")
                        nc.tensor.matmul(bc[:], ones4k[:, 0:64], r_bf[:],
                                         start=True, stop=True)
                        nc.scalar.copy(bc_sb[:], bc[:])
                        nc.vector.tensor_mul(
                            out_all[p][64 * h:64 * (h + 1), js],
                            av[h][0:64, :], bc_sb[:])

            # ---------- tail: proj + residual + feat + output ----------
            for m in range(2):
                lo = 1024 + 128 * m
                for j in range(2):
                    js = slice(512 * j, 512 * (j + 1))
                    pp = attps.tile([128, 512], F32, tag=f"at{m}", name="pp")
                    for k in range(2):
                        nc.tensor.matmul(pp[:], wsl(k, lo, lo + 128),
                                         out_all[k][:, js], start=(k == 0), stop=(k == 1))
                    nc.vector.tensor_add(
                        tok_upd[m][:, js], pp[:],
                        tok_new[m][:, bass.ds(dve_off, Q)][:, js])
            for m in range(2):
                lo = 1280 + 128 * m
                o_sb = normp.tile([128, Q], F32, tag="o_sb", name="o_sb")
                for j in range(2):
                    js = slice(512 * j, 512 * (j + 1))
                    pf = attps.tile([128, 512], F32, tag=f"at{m}", name="pf")
                    for k in range(2):
                        nc.tensor.matmul(pf[:], wsl(k, lo, lo + 128),
                                         tok_upd[k][:, js], start=(k == 0), stop=False)
                    nc.tensor.matmul(pf[:], bfeat_t[0:1, 128 * m:128 * (m + 1)],
                                     ones4k[:, js], start=False, stop=True)
                    nc.vector.scalar_tensor_tensor(
                        o_sb[:, js], pf[:], 0.2, xsl[m][:, js],
                        AluOp.mult, AluOp.add)
                nc.sync.dma_start(out_ext.ap()[128 * m:128 * (m + 1), :], o_sb[:])

            if dbg:
                for k in range(2):
                    r = slice(128 * k, 128 * (k + 1))
                    nc.sync.dma_start(dbg["d_token_new"].ap()[r, :], tok_new[k][:])
                    nc.sync.dma_start(dbg["d_out_all"].ap()[r, :], out_all[k][:])
                nc.sync.dma_start(dbg["d_kp0"].ap(), kp[0][:])
                nc.sync.dma_start(dbg["d_v1"].ap(),
                                  v1[:].rearrange("p c e -> p (c e)"))
                nc.sync.dma_start(dbg["d_q0"].ap(), qsb[0][:])


def pack_inputs(x, w_tok, b_tok, w_red, b_red, w_dil, b_dil, w_fuse, b_fuse,
                w_qkv, w_proj, w_feat, b_feat):
    """Host-side packing: full inputs -> list of 8 per-core input maps."""
    common = {}
    wcat = np.concatenate([
        np.ascontiguousarray(w_tok.T),
        np.ascontiguousarray(w_qkv[0:256].T),
        np.ascontiguousarray(w_qkv[256:512].T),
        np.ascontiguousarray(w_qkv[512:768].T),
        np.ascontiguousarray(w_proj.T),
        np.ascontiguousarray(w_feat),
        np.ascontiguousarray(w_red.T),
    ], axis=1).astype(BF)
    common["wcat"] = wcat

    # dil-conv lhsT: rows (K) = kh*42 + ci*3 + kw (per-kh im2col DMA order);
    # cols (M) = c*9 + j (feats row layout)
    w9 = [np.transpose(w_dil[d], (2, 1, 3, 0)).reshape(126, CMID)
          for d in range(3)]

    def mkL(blocks, bias):
        L = np.zeros((127, 126), np.float32)
        for j, (w, b) in enumerate(zip(blocks, bias)):
            if w is None:
                continue
            L[0:126, 14 * j:14 * (j + 1)] = w
            L[126, 14 * j:14 * (j + 1)] = b
        return L.astype(BF)

    L1a = mkL([w9[0]] * 6 + [None] * 3, [b_dil[0]] * 6 + [None] * 3)
    L1b = mkL([None] * 6 + [w9[1]] * 3, [None] * 6 + [b_dil[1]] * 3)
    L2a = mkL([w9[1]] * 3 + [None] * 6, [b_dil[1]] * 3 + [None] * 6)
    L2b = mkL([None] * 3 + [w9[2]] * 6, [None] * 3 + [b_dil[2]] * 6)
    common["Lcat"] = np.concatenate([L1a, L1b, L2a, L2b], axis=1)

    wf1 = np.ascontiguousarray(w_fuse[:, 0:126].T)
    wf2 = np.ascontiguousarray(w_fuse[:, 126:252].T)
    wf2 = np.vstack([wf2, np.asarray(b_fuse)[None, :]])
    wfusecat = np.zeros((127, 512), np.float32)
    wfusecat[:, 0:256] = wf2
    wfusecat[0:126, 256:512] = wf1
    common["wfusecat"] = wfusecat.astype(BF)

    bcat = np.zeros((128, 3), np.float32)
    bcat[:, 0] = b_tok[0:128]
    bcat[:, 1] = b_tok[128:256]
    bcat[0:CMID, 2] = b_red
    common["bcat"] = bcat
    common["bfeat_bf"] = np.asarray(b_feat).reshape(1, 256).astype(BF)

    in_maps = []
    for c in range(8):
        b, s = c // 4, c % 4
        m = dict(common)
        xb = x[b].reshape(256, N_TOK)
        m["xbf"] = np.ascontiguousarray(xb).astype(BF)
        m["xslice"] = np.ascontiguousarray(xb[:, s * Q:(s + 1) * Q], np.float32)
        m["qoff"] = np.array([[s * Q]], np.int32)
        in_maps.append(m)
    return in_maps


_NC_CACHE = {}


def get_nc(debug=False):
    if debug not in _NC_CACHE:
        _NC_CACHE[debug] = build(debug)
    return _NC_CACHE[debug]


def run(in_maps, debug=False, trace=False):
    nc = get_nc(debug)
    return run_bass_kernel_spmd(nc, in_maps, core_ids=list(range(8)), trace=trace)


def assemble(results, x):
    out = np.empty((2, 256, N_TOK), np.float32)
    for c, res in enumerate(results):
        b, s = c // 4, c % 4
        out[b, :, s * Q:(s + 1) * Q] = res["out"]
    return out.reshape(2, 256, 64, 64)


def kernel(**inputs):
    in_maps = pack_inputs(**inputs)
    r = run(in_maps)
    return assemble(r.results, inputs["x"])
